# revision 54
# baseline (speedup 1.0000x reference)
"""Bass/Tile Trainium2 kernel for CrossPositionalAttention.

Reference math (per batch element b):
    M = F @ W_M; N = F @ W_N; V = F @ W_V          # [T, C] each, T=2048, C=64
    S = softmax(M @ N^T, axis=-1)                  # [T, T]
    out = S @ V + F

Sharding: data-parallel over batch. B=8 == n_cores=8, so core i computes
batch element i end-to-end (no collectives); kernel() shards/gathers on host.

Key structure (per core, P=128 partitions):
  Weight fusion: scores = M N^T = F (W_M W_N^T) F^T, so the host passes
    G = W_M W_N^T and the kernel computes P^T = G^T F^T once ([64, T]);
    scores^T tiles are then F_T_blk^T @ P^T -- no N projection at all.
  fp16 operands: F^T and P^T are stored fp16 (1 cyc/col PE streaming like
    bf16, but 10 mantissa bits; values are O(10) so range is safe).  expS
    and V are bf16 (exp(s-40) reaches e^29, beyond fp16 range).
  Mainloop (per q-chunk of 512, per kp pair of k-blocks): two k=64 scores
    matmuls, row-packed at array rows 0-63/64-127 (concurrent, and each
    tile's LDWEIGHTS overlaps the other row-group's matmul) -> PSUM
    [128, 1024]; exp -> bf16 expS; PV matmuls accumulate [66, 512] f32:
    V rows plus a ones-column that yields the softmax denominator.
  Exp split across TWO engines: 6 of 8 iterations per chunk on ACT
    (exp, bias -40; softmax is shift-invariant and scores stay in fp32
    range), and iterations kp 1/3 on the DVE via a Schraudolph
    fast-exp: the bf16 bit pattern of exp(s-40) is one affine
    tensor_scalar, round((s+48.004)*184.665), written as uint16 (the
    f32->u16 convert rounds AND saturates negatives to 0, zeroing
    scores < -48 whose true exp is < 1e-38 anyway).
  PV lags scores by TWO iterations (a lag-2 software pipeline carried
    ACROSS chunk boundaries, pv PSUM single-buffered): the next
    iteration's scores always sit ahead of the exp-blocked PV pair on
    the FIFO PE queue, so neither exp engine is head-of-line blocked.
  HAM clock gate: sustained PE micro-idle demotes the PE to 1.2 GHz and
    it sticks at half clock (mid-kernel re-promotion is unreliable).  An
    UNINTERRUPTED ~3.5us warm-up burst of dummy matmuls (a DMA-wait
    inside the burst would reset the activity window) trips the gate to
    K=8/8; phase-A group 0 runs right after it and groups 1-3 spread
    over qc=0 iterations, all bridged with dummies (transpose-mode does
    not count as PE activity); the mainloop keeps the PE ~95% busy with
    real work so the gate never demotes.
  Per-chunk epilogue (transpose pv -> [128,66], out = pv[:, :64] *
    recip(pv[:, 64]) + F) is interleaved one block per iteration into the
    NEXT q-chunk (kp 2/4/6/7) so the PE never bursts at chunk
    boundaries; the last chunk pipelines per-block across engines with
    half-chunk DMAs on both queues (each dma_start costs ~650ns of
    sequencer issue time).
"""

import numpy as np

import concourse.bacc as bacc
import concourse.bass as bass
import concourse.tile as tile
from concourse import mybir
from concourse.bass_utils import run_bass_kernel_spmd
from concourse.masks import make_identity

B, T, C = 8, 2048, 64
P = 128
NBLK = T // P          # 16 k-blocks (and q-blocks) of 128
QCHUNK = 512           # moving-operand free dim per matmul
NQC = T // QCHUNK      # 4 q-chunks
F32 = mybir.dt.float32
BF16 = mybir.dt.bfloat16
FP16 = mybir.dt.float16
F32R = mybir.dt.float32r
EXP_BIAS = -40.0       # constant softmax shift (cancels in the normalization)
VPAD = 66              # V tile free dim: 64 V cols + ones col + pad (even)
NWARM = 8              # uninterrupted head warm-up burst (8 x ~427ns cold)


def build_nc() -> bass.Bass:
    nc = bacc.Bacc()
    F_h = nc.declare_dram_parameter("F", [T, C], F32, isOutput=False)
    G_h = nc.declare_dram_parameter("G", [C, C], F32, isOutput=False)
    Wv_h = nc.declare_dram_parameter("W_V", [C, C], F32, isOutput=False)
    out_h = nc.declare_dram_parameter("out", [T, C], F32, isOutput=True)

    # [T, C] viewed as [128, 16, C]: partition p, block n -> row n*128 + p
    F_view = F_h[:, :].rearrange("(n p) c -> p n c", p=P)
    out_view = out_h[:, :].rearrange("(n p) c -> p n c", p=P)

    with tile.TileContext(nc) as tc:
        with (
            tc.tile_pool(name="const", bufs=1) as const_pool,
            tc.tile_pool(name="persist", bufs=1) as persist,
            # PSUM: sc 2x2 banks + mix 3x1 + pv 1x1 = 8 banks.  pv can be
            # single-buffered (its drain CAST completes a full iteration
            # before the next chunk's first PV); mix bufs=3 keeps the
            # prep/filler/epilogue rotation from stalling the PE on the
            # DVE casts of the tile two allocations back.
            tc.tile_pool(name="mix_ps", bufs=3, space="PSUM") as mix_ps,
            tc.tile_pool(name="sc_ps", bufs=2, space="PSUM") as sc_pool,
            tc.tile_pool(name="pv_ps", bufs=1, space="PSUM") as pv_pool,
            tc.tile_pool(name="work", bufs=6) as work,
            tc.tile_pool(name="ep", bufs=4) as ep,
            tc.tile_pool(name="opool", bufs=2) as opool,
        ):
            # warm tile on DVE (fast) so the burst starts ASAP
            warm = const_pool.tile([P, P + QCHUNK], BF16, tag="warm")
            nc.vector.memset(warm, 0.25)

            # small weights first (tiny), then F chunked across both HWDGE
            # queues so phase-A groups can start as each chunk lands
            Wstage = const_pool.tile([C, 2, C], F32, tag="wstage")
            nc.sync.dma_start(out=Wstage[:, 0, :], in_=G_h[:, :])
            nc.sync.dma_start(out=Wstage[:, 1, :], in_=Wv_h[:, :])

            F_sb = persist.tile([P, NBLK, C], F32, tag="fsb")
            nc.scalar.dma_start(out=F_sb[:, 0:2, :], in_=F_view[:, 0:2, :])
            nc.sync.dma_start(out=F_sb[:, 2:4, :], in_=F_view[:, 2:4, :])
            nc.scalar.dma_start(out=F_sb[:, 4:8, :], in_=F_view[:, 4:8, :])
            nc.sync.dma_start(out=F_sb[:, 8:12, :], in_=F_view[:, 8:12, :])
            nc.scalar.dma_start(out=F_sb[:, 12:16, :], in_=F_view[:, 12:16, :])

            # G duplicated along the free dim: one P^T matmul then fills
            # both partition halves of PT (for row-packed scores tiles)
            G2 = const_pool.tile([C, P], FP16, tag="g2")
            Wv_sb = const_pool.tile([C, C], FP16, tag="wv")
            for h in range(2):
                nc.vector.tensor_copy(G2[:, h * C : (h + 1) * C], Wstage[:, 0, :])
            nc.vector.tensor_copy(Wv_sb, Wstage[:, 1, :])

            ident = const_pool.tile([P, P], F32, tag="ident")
            make_identity(nc, ident)
            ident_r = const_pool.tile([P, P], F32R, tag="identr")
            nc.vector.tensor_copy(ident_r, ident)
            ident16 = const_pool.tile([P, P], FP16, tag="ident16")
            nc.vector.tensor_copy(ident16, ident)
            # fp16 copy of F: PE transposes stream fp16 at 1 cyc/row
            # (f32 is 2 cyc/row), halving phase-A transpose time
            F16_sb = persist.tile([P, NBLK, C], FP16, tag="f16sb")
            for fq in range(4):
                nc.vector.tensor_copy(
                    F16_sb[:, 4 * fq : 4 * fq + 4, :],
                    F_sb[:, 4 * fq : 4 * fq + 4, :],
                )

            exp_bias = const_pool.tile([P, 1], F32, tag="expbias")
            nc.vector.memset(exp_bias, EXP_BIAS)
            # preload the exp ACT table while DMAs land (issued on the scalar
            # queue after its F dma_starts; ~2.7us table load off critical path)
            tbl_dummy = const_pool.tile([P, 1], F32, tag="tbldummy")
            nc.scalar.activation(
                tbl_dummy, exp_bias, mybir.ActivationFunctionType.Exp
            )

            # F^T and P^T duplicated into both partition halves so scores
            # can row-pack two k=64 tiles (tile_position h*64) -- packing
            # runs the pair concurrently AND lets LDWEIGHTS of one tile
            # overlap the other tile's matmul (different row groups)
            F_T = persist.tile([P, T], FP16, tag="ft")
            PT = persist.tile([P, T], FP16, tag="pt")
            # PV path in bf16 (fp16 cannot hold exp(s-40) up to e^29)
            V_sb = persist.tile([P, NBLK, VPAD], BF16, tag="vsb")
            # ones col 64 of every V block -> softmax denominator via PV
            nc.vector.memset(V_sb[:, :, C:VPAD], 1.0)

            def warm_mm(n=QCHUNK):
                # dummy bf16 matmul: counts as real PE activity for the
                # HAM clock gate (transposes don't)
                wps = mix_ps.tile([P, QCHUNK], F32, tag="mix", name="wps")
                nc.tensor.matmul(
                    wps[:, 0:n],
                    lhsT=warm[:, 0:P],
                    rhs=warm[:, P : P + n],
                    start=True,
                    stop=True,
                )

            def prep_t2(g, half):
                """F^T transposes for k-blocks 4g+2*half..+2 (both halves)."""
                tp = mix_ps.tile([C, 2, P], FP16, tag="mix", name="tp")
                n0 = 4 * g + 2 * half
                for i in range(2):
                    nc.tensor.transpose(
                        tp[:, i, :], F16_sb[:, n0 + i, :], ident16
                    )
                # transpose-mode does NOT count as PE activity for the HAM
                # clock gate -- bridge the transpose window with a real
                # matmul or the gate demotes to half clock here
                warm_mm()
                # PSUM f32 -> SBUF fp16 copies perform the rounding;
                # second copy partition-shifts into rows 64-127
                sl2 = slice(n0 * P, (n0 + 2) * P)
                nc.vector.tensor_copy(F_T[0:C, sl2], tp)
                nc.vector.tensor_copy(F_T[C:P, sl2], tp)

            def prep_t(g):
                prep_t2(g, 0)
                prep_t2(g, 1)

            def prep_p(g):
                """P^T = G^T F^T chunk g (both halves via duplicated G2)."""
                sl = slice(g * QCHUNK, (g + 1) * QCHUNK)
                pp = mix_ps.tile([P, QCHUNK], F32, tag="mix", name="pp")
                nc.tensor.matmul(
                    pp, lhsT=G2, rhs=F_T[0:C, sl], start=True, stop=True
                )
                nc.vector.tensor_copy(PT[:, sl], pp)

            def prep_v(g):
                """V = F W_V blocks 4g..4g+3 (bf16 out)."""
                vp = mix_ps.tile([P, 4, C], F32, tag="mix", name="vp")
                for i in range(4):
                    n = 4 * g + i
                    nc.tensor.matmul(
                        vp[:, i, :],
                        lhsT=F_T[0:C, n * P : (n + 1) * P],
                        rhs=Wv_sb,
                        start=True,
                        stop=True,
                    )
                nc.vector.tensor_copy(V_sb[:, 4 * g : 4 * g + 4, 0:C], vp)

            # uninterrupted warm-up burst (~3.5us cold) trips the HAM gate;
            # by its end the F DMAs have landed, so groups 0/1 run warm
            for _ in range(NWARM):
                warm_mm()
            # group 0 in the head; groups 1-3 interleave into qc=0 (the
            # exp pipeline starts ~4us earlier -- scores(0,kp) only needs
            # group kp//2, and qc0's exps are all-ACT at ~1147ns pace
            # while the PE runs preps in the slack)
            prep_t(0)
            prep_p(0)
            prep_v(0)

            def ep_block(pv_sb_p, o_sb_p, qcp, j):
                """One 128-q block of the chunk-qcp epilogue (steady chunks)."""
                qb = qcp * (QCHUNK // P) + j
                trr = mix_ps.tile([P, VPAD], F32R, tag="mix", name="trr")
                nc.tensor.transpose(
                    trr,
                    pv_sb_p[:, j * P : (j + 1) * P],
                    ident_r[0:VPAD, 0:VPAD],
                )
                tr = trr.bitcast(F32)
                rcp = ep.tile([P, 1], F32, tag="rcp")
                nc.vector.reciprocal(rcp, tr[:, C : C + 1])
                nc.vector.tensor_scalar_mul(o_sb_p[:, j, :], tr[:, 0:C], rcp)
                nc.vector.tensor_add(
                    o_sb_p[:, j, :], o_sb_p[:, j, :], F_sb[:, qb, :]
                )
                if j == 3:
                    nc.sync.dma_start(
                        out=out_view[:, qcp * NQC : (qcp + 1) * NQC, :],
                        in_=o_sb_p,
                    )

            prev_ep = None  # (pv_sb, o_sb, qc) of the chunk awaiting epilogue
            pend = []       # [(qc, kp, expS)] awaiting PV -- lag-2 pipeline
            pv_tiles = {}   # qc -> its PSUM accumulator (single bank)

            def flush_pv(p):
                """Emit the PV pair for pend entry p; drain its chunk on
                the last pair.  PV lags scores by TWO iterations so the
                next iteration's scores sit AHEAD of it on the FIFO PE
                queue -- during a DVE-exp window the ACT's next exp then
                waits only on scores, not on scores + a blocked PV."""
                nonlocal prev_ep
                qcp, kpp, eS = p
                if qcp not in pv_tiles:
                    pv_tiles[qcp] = pv_pool.tile(
                        [VPAD, QCHUNK], F32, tag="pv", name="pvt"
                    )
                pvt = pv_tiles[qcp]
                lastp = kpp == NBLK // 2 - 1
                for h in range(2):
                    nc.tensor.matmul(
                        pvt,
                        lhsT=V_sb[:, 2 * kpp + h, :],
                        rhs=eS[:, h * QCHUNK : (h + 1) * QCHUNK],
                        start=(kpp == 0 and h == 0),
                        stop=(lastp and h == 1),
                    )
                if lastp:
                    # chunk qcp complete: drain PSUM and queue its epilogue
                    pv_sb = ep.tile([VPAD, QCHUNK], F32R, tag="pvsb")
                    nc.vector.tensor_copy(pv_sb, pvt)
                    o_sb = opool.tile([P, NQC, C], F32, tag="osb")
                    prev_ep = (pv_sb, o_sb, qcp)

            for qc in range(NQC):
                qsl = slice(qc * QCHUNK, (qc + 1) * QCHUNK)
                for kp in range(NBLK // 2):
                    sc_ps = sc_pool.tile([P, 2 * QCHUNK], F32, tag="sc")
                    # scores^T for k-blocks 2kp / 2kp+1, row-packed: the two
                    # k=64 tiles occupy array rows 0-63 / 64-127 and run
                    # concurrently (lhsT/rhs partition halves must match
                    # the tile_position row offset).  Scores go FIRST in
                    # the iteration: everything else (preps, epilogue,
                    # fillers, the exp(kp-1)-blocked PV pair) sits BEHIND
                    # them on the FIFO PE queue, so the exp pipeline is
                    # never head-of-line blocked.
                    for h, kblk in ((0, 2 * kp), (1, 2 * kp + 1)):
                        rows = slice(h * C, h * C + C)
                        ksl = slice(kblk * P, (kblk + 1) * P)
                        bank = slice(h * QCHUNK, (h + 1) * QCHUNK)
                        nc.tensor.matmul(
                            sc_ps[:, bank],
                            lhsT=F_T[rows, ksl],
                            rhs=PT[rows, qsl],
                            start=True,
                            stop=True,
                            tile_position=(h * C, 0),
                        )
                    expS = work.tile([P, 2 * QCHUNK], BF16, tag="exps")
                    # DVE fast-exps only mid-chunk (kp 1/3): a DVE exp near
                    # the chunk end gates the next chunk's first scores
                    # through the sc-buffer WAR, and in qc0 the DVE queue
                    # is congested with phase-A casts -- those all stay ACT
                    if qc > 0 and kp in (1, 3):
                        # DVE fast-exp (Schraudolph): the bf16 bit pattern of
                        # exp(s-40) is round((s-40)*184.665 + 16251.25) --
                        # computed as one affine tensor_scalar with uint16
                        # output (f32->u16 convert rounds AND saturates
                        # negatives to 0, which zeroes scores < -48 whose
                        # true exp(s-40) < 1e-38 anyway).  ~2-3% weight
                        # error that largely cancels in the softmax
                        # normalization; offloading 2 of 8 exps per chunk
                        # takes ACT off the critical path.
                        nc.vector.tensor_scalar(
                            out=expS.bitcast(mybir.dt.uint16),
                            in0=sc_ps,
                            scalar1=48.003983,
                            scalar2=184.6649652,
                            op0=mybir.AluOpType.add,
                            op1=mybir.AluOpType.mult,
                        )
                    elif qc == NQC - 1 and kp == NBLK // 2 - 1:
                        # tail: split the last exp so the final PV pair
                        # starts after the first half (subtile deps)
                        for h in range(2):
                            hsl = slice(h * QCHUNK, (h + 1) * QCHUNK)
                            nc.scalar.activation(
                                expS[:, hsl],
                                sc_ps[:, hsl],
                                mybir.ActivationFunctionType.Exp,
                                bias=exp_bias,
                                scale=1.0,
                            )
                    else:
                        nc.scalar.activation(
                            expS,
                            sc_ps,
                            mybir.ActivationFunctionType.Exp,
                            bias=exp_bias,
                            scale=1.0,
                        )
                    pend.append((qc, kp, expS))
                    # phase-A groups 1-3 spread over qc=0: transposes one
                    # iteration, P^T/V the next (each fits the ~800ns PE
                    # slack under the ACT exp pace)
                    if qc == 0:
                        if kp in (1, 3, 5):
                            prep_t(kp // 2 + 1)
                        elif kp in (2, 4, 6):
                            prep_p(kp // 2)
                            prep_v(kp // 2)
                    # lag-2 PV: emit the pair from two iterations ago
                    if len(pend) > 2:
                        flush_pv(pend.pop(0))
                    # previous chunk's epilogue: one block per iteration
                    # (kp 2,4,6,7 -- its drain lands at kp1; kp 1/3/5 have
                    # their DVE slot busy with the fast-exp)
                    if prev_ep is not None and kp in (2, 4, 6, 7):
                        j = {2: 0, 4: 1, 6: 2, 7: 3}[kp]
                        ep_block(prev_ep[0], prev_ep[1], prev_ep[2], j)
                        if j == 3:
                            prev_ep = None
                    # PE fillers only in the thin first two iterations
                    # (no PV pairs in flight yet); elsewhere the PE is
                    # saturated with real work
                    if qc == 0 and kp in (0, 1):
                        warm_mm()
                        if kp == 0:
                            warm_mm()

            # ---- tail: flush the two pending PV pairs of the last chunk ----
            qc = NQC - 1
            flush_pv(pend.pop(0))          # (qc3, kp6) -- DVE exp, ready
            qcp, kpp, eS = pend.pop(0)     # (qc3, kp7) -- split-half exp
            pvt = pv_tiles[qcp]
            for h in range(2):
                nc.tensor.matmul(
                    pvt,
                    lhsT=V_sb[:, 2 * kpp + h, :],
                    rhs=eS[:, h * QCHUNK : (h + 1) * QCHUNK],
                    start=False,
                    stop=(h == 1),
                )
            pv_sb = ep.tile([VPAD, QCHUNK], F32R, tag="pvsb")
            # quarter the copy so the first transpose starts after 1/4 of
            # the data is in SBUF; quarters alternate DVE/ACT (ACT is
            # genuinely free after the last exp) so they land ~2x sooner
            for q4 in range(4):
                sl4 = slice(q4 * P, (q4 + 1) * P)
                if q4 % 2 == 0:
                    nc.vector.tensor_copy(pv_sb[:, sl4], pvt[:, sl4])
                else:
                    nc.scalar.copy(pv_sb[:, sl4], pvt[:, sl4])
            o_sb = opool.tile([P, NQC, C], F32, tag="osb")
            # last chunk pipelines per-block across engines
            for j in range(QCHUNK // P):
                qb = qc * (QCHUNK // P) + j
                trr = mix_ps.tile([P, VPAD], F32R, tag="mix", name="trr")
                nc.tensor.transpose(
                    trr,
                    pv_sb[:, j * P : (j + 1) * P],
                    ident_r[0:VPAD, 0:VPAD],
                )
                tr = trr.bitcast(F32)
                rcp = ep.tile([P, 1], F32, tag="rcp")
                nc.vector.reciprocal(rcp, tr[:, C : C + 1])
                # spread the chain across three engines -- muls on
                # Scalar (idle after the last exp) + DVE, residual
                # adds on GpSimd (SBUF-only, allowed)
                if j % 2 == 0:
                    nc.scalar.activation(
                        o_sb[:, j, :],
                        tr[:, 0:C],
                        mybir.ActivationFunctionType.Copy,
                        scale=rcp,
                    )
                else:
                    nc.vector.tensor_scalar_mul(o_sb[:, j, :], tr[:, 0:C], rcp)
                if j == 3:
                    # last block: DVE add -- the GpSimd queue still holds
                    # the other blocks' adds and would gate the final DMA
                    nc.vector.tensor_add(
                        o_sb[:, j, :], o_sb[:, j, :], F_sb[:, qb, :]
                    )
                else:
                    nc.gpsimd.tensor_tensor(
                        out=o_sb[:, j, :],
                        in0=o_sb[:, j, :],
                        in1=F_sb[:, qb, :],
                        op=mybir.AluOpType.add,
                    )
                # two half-chunk DMAs: the first issues (~650ns sequencer
                # cost) and flies while blocks 2/3 still compute; the
                # second goes on the scalar queue so the issues overlap
                if j == 1:
                    nc.sync.dma_start(
                        out=out_view[:, qc * NQC : qc * NQC + 2, :],
                        in_=o_sb[:, 0:2, :],
                    )
                elif j == 3:
                    nc.scalar.dma_start(
                        out=out_view[:, qc * NQC + 2 : qc * NQC + 4, :],
                        in_=o_sb[:, 2:4, :],
                    )

    nc.finalize()
    return nc


_NC_CACHE = None


def _get_nc() -> bass.Bass:
    global _NC_CACHE
    if _NC_CACHE is None:
        _NC_CACHE = build_nc()
    return _NC_CACHE


def run_spmd(F, W_M, W_N, W_V, **kwargs):
    """Run the SPMD kernel; returns the BassKernelResults (for profiling)."""
    nc = _get_nc()
    G = np.ascontiguousarray(
        W_M.astype(np.float32) @ W_N.astype(np.float32).T
    )
    in_maps = [
        {
            "F": np.ascontiguousarray(F[i], dtype=np.float32),
            "G": G,
            "W_V": np.ascontiguousarray(W_V, dtype=np.float32),
        }
        for i in range(B)
    ]
    return run_bass_kernel_spmd(nc, in_maps, core_ids=list(range(B)), **kwargs)


def kernel(F, W_M, W_N, W_V):
    res = run_spmd(F, W_M, W_N, W_V)
    return np.stack([r["out"] for r in res.results]).astype(np.float32)
